# revision 1
# baseline (speedup 1.0000x reference)
"""Trainium2 Bass kernel for RoPE multi-head attention (B=2, T=2048, D=1024, H=16).

Sharding: tensor-parallel over heads. Each of the 8 cores owns 2 heads
(128 of the 1024 qkv dims):
  - QKV projections: every core holds X^T (replicated) and its 128-column
    slice of Wq/Wk/Wv; computes Q^T, K^T (rotated via RoPE) and V for its
    heads only.
  - Attention: scores computed transposed (S^T = K @ Q^T, [keys, toks]) so
    that exp(S^T) can feed the P@V matmul directly as the moving operand.
    Causality: only lower-triangular key/tok blocks are computed; the
    diagonal 128x128 block is masked with a 0/1 triangle after exp.
    The softmax denominator is produced by the same PV matmul via a ones
    column appended to V (stationary operand is [keys, 65]).
  - Output projection: an AllToAll redistributes attention outputs from
    head-sharded to token-sharded; each core then computes its 512-token
    slice of the final output against the full Wo.
Host side only reshapes / transposes / casts / shards; all arithmetic
(including the RoPE sin/cos tables, computed from token_positions on the
ScalarEngine) happens on device.
"""

import sys
from contextlib import ExitStack

for _p in ("/opt/trn_rl_repo",):
    if _p not in sys.path:
        sys.path.append(_p)

import numpy as np
import ml_dtypes

import concourse.bacc as bacc
import concourse.bass as bass
import concourse.mybir as mybir
import concourse.tile as tile
from concourse.bass_utils import run_bass_kernel_spmd

# Problem constants (hardcoded per harness contract).
B, T, D, H, DH = 2, 2048, 1024, 16, 64
NCORES = 8
HPC = H // NCORES          # heads per core = 2
TOK = B * T                # 4096 flattened tokens
THETA = 10000.0
SCALE = 1.0 / 8.0          # 1/sqrt(DH)
KS = D // 128              # 8 contraction slices
NKT = T // 128             # 16 key tiles per batch
NCH = T // 512             # 4 tok chunks (512) per batch
TPC = TOK // NCORES        # 512 tokens per core for the output projection

BF16 = mybir.dt.bfloat16
F32 = mybir.dt.float32


def build_nc(skip_collective=False, dummy=False):
    nc = bacc.Bacc(
        "TRN2",
        target_bir_lowering=False,
        debug=False,
        num_devices=NCORES,
    )

    # ---- kernel I/O ----
    xt_d = nc.dram_tensor("xt", [D, TOK], BF16, kind="ExternalInput")
    wqt_d = nc.dram_tensor("wqt", [D, 128], BF16, kind="ExternalInput")
    wkt_d = nc.dram_tensor("wkt", [D, 128], BF16, kind="ExternalInput")
    wvt_d = nc.dram_tensor("wvt", [D, 128], BF16, kind="ExternalInput")
    wot_d = nc.dram_tensor("wot", [D, D], BF16, kind="ExternalInput")
    posf_d = nc.dram_tensor("posf", [1, T], F32, kind="ExternalInput")
    out_d = nc.dram_tensor("out", [D, TPC], F32, kind="ExternalOutput")

    # ---- compile-time constants ----
    inv_freq = (1.0 / THETA ** (np.arange(DH // 2, dtype=np.float64) / (DH // 2)))
    invf4 = np.tile(inv_freq.astype(np.float32), 4)[None, :]  # [1, 128]
    invf_d = nc.inline_tensor(invf4, "invf")
    # tri[j, i] = 1 iff key j <= tok i (keeps lower-triangular attention).
    tri = np.triu(np.ones((128, 128), np.float32)).astype(ml_dtypes.bfloat16)
    tri_d = nc.inline_tensor(tri, "tri")

    if dummy:
        # identical I/O signature, near-empty body: used to measure the
        # fixed dispatch/NEFF-launch overhead so it can be subtracted
        with tile.TileContext(nc) as tc, ExitStack() as ctx:
            sp = ctx.enter_context(tc.tile_pool(name="sp", bufs=1))
            z = sp.tile([128, 16], F32, tag="z")
            nc.vector.memset(z, 0.0)
            nc.sync.dma_start(out=out_d[0:128, 0:16], in_=z)
        nc.compile()
        return nc

    with tile.TileContext(nc) as tc, ExitStack() as ctx:
        singles = ctx.enter_context(tc.tile_pool(name="singles", bufs=1))
        tmp = ctx.enter_context(tc.tile_pool(name="tmp", bufs=2))
        ppool = ctx.enter_context(tc.tile_pool(name="ppool", bufs=6))
        apool = ctx.enter_context(tc.tile_pool(name="apool", bufs=4))
        ps_main = ctx.enter_context(tc.tile_pool(name="ps_main", bufs=2, space="PSUM"))
        ps_pv = ctx.enter_context(tc.tile_pool(name="ps_pv", bufs=4, space="PSUM"))
        dpool = ctx.enter_context(tc.tile_pool(name="dram", bufs=1, space="DRAM"))

        # ---- persistent SBUF tensors ----
        # small inputs first so they don't queue behind the 8MB X^T load
        posf = singles.tile([1, T], F32, tag="posf")
        nc.sync.dma_start(out=posf, in_=posf_d.ap())
        invf = singles.tile([1, 128], F32, tag="invf")
        nc.sync.dma_start(out=invf, in_=invf_d.ap())
        tri_sb = singles.tile([128, 128], BF16, tag="tri")
        nc.sync.dma_start(out=tri_sb, in_=tri_d.ap())
        wq = singles.tile([128, KS, 128], BF16, tag="wq")
        nc.sync.dma_start(out=wq, in_=wqt_d.ap().rearrange("(k p) m -> p k m", p=128))
        wk = singles.tile([128, KS, 128], BF16, tag="wk")
        nc.sync.dma_start(out=wk, in_=wkt_d.ap().rearrange("(k p) m -> p k m", p=128))
        wv = singles.tile([128, KS, 128], BF16, tag="wv")
        nc.sync.dma_start(out=wv, in_=wvt_d.ap().rearrange("(k p) m -> p k m", p=128))
        xt = singles.tile([128, KS, TOK], BF16, tag="xt")
        xt_r = xt_d.ap().rearrange("(k p) t -> p k t", p=128)
        for k in range(KS):
            nc.sync.dma_start(out=xt[:, k, :], in_=xt_r[:, k, :])
        wo = singles.tile([128, KS, D], BF16, tag="wo")
        nc.sync.dma_start(out=wo, in_=wot_d.ap().rearrange("(k p) f -> p k f", p=128))

        qt = singles.tile([128, TOK], BF16, tag="qt")     # rotated Q^T
        kt = singles.tile([128, TOK], BF16, tag="kt")     # rotated K^T
        # V per (batch, keytile): [keys=128, 65*HPC]; col 64/129 = ones.
        vsb = singles.tile([128, B, NKT, 65 * HPC], BF16, tag="vsb")
        nc.vector.memset(vsb[:, :, :, 64:65], 1.0)
        nc.vector.memset(vsb[:, :, :, 129:130], 1.0)
        # ones row at partition 64 (same partition the PV rowsum lands on),
        # used to broadcast 1/rowsum across the 64 head dims via a K=1 matmul
        ones65 = singles.tile([65, 64], F32, tag="ones65")
        nc.vector.memset(ones65[64:65, :], 1.0)

        # cos table replicated over 4 row-blocks; sin table with sign baked
        # per half: rows [0:32] -sin, [32:64] +sin, then repeated.
        cs4 = singles.tile([128, T], F32, tag="cs4")
        sns4 = singles.tile([128, T], F32, tag="sns4")

        # ---- RoPE tables: ang = pos * inv_freq, sin/cos on ScalarE ----
        # ScalarE Sin needs args in [-pi, pi]. Range-reduce with the fp32
        # magic-number round: rn(x) = (x + 1.5*2^23) - 1.5*2^23, so
        # red = ang - 2pi*rn(ang/2pi) lands in [-pi, pi] (clamped for safety).
        tau = float(2 * np.pi)
        magic = float(1.5 * 2 ** 23)

        def reduced_sin(dst, src_ap):
            qm = tmp.tile([128, 512], F32, tag="ra", name="qm", bufs=1)
            nc.vector.tensor_scalar(
                out=qm, in0=src_ap, scalar1=1.0 / tau, scalar2=magic,
                op0=mybir.AluOpType.mult, op1=mybir.AluOpType.add,
            )
            qr = tmp.tile([128, 512], F32, tag="rb", name="qr", bufs=1)
            nc.vector.tensor_scalar(
                out=qr, in0=qm, scalar1=magic, scalar2=None,
                op0=mybir.AluOpType.subtract,
            )
            red = tmp.tile([128, 512], F32, tag="rc", name="red", bufs=1)
            nc.vector.scalar_tensor_tensor(
                out=red, in0=qr, scalar=-tau, in1=src_ap,
                op0=mybir.AluOpType.mult, op1=mybir.AluOpType.add,
            )
            redc = tmp.tile([128, 512], F32, tag="rd", name="redc", bufs=1)
            nc.vector.tensor_scalar(
                out=redc, in0=red, scalar1=float(np.pi), scalar2=float(-np.pi),
                op0=mybir.AluOpType.min, op1=mybir.AluOpType.max,
            )
            nc.scalar.activation(
                out=dst, in_=redc, func=mybir.ActivationFunctionType.Sin
            )

        for q4 in range(T // 512):
            ps_a = ps_main.tile([128, 512], F32, tag="big")
            nc.tensor.matmul(
                ps_a, invf, posf[:, q4 * 512:(q4 + 1) * 512], start=True, stop=True
            )
            sl = slice(q4 * 512, (q4 + 1) * 512)
            reduced_sin(sns4[:, sl], ps_a)
            angc = tmp.tile([128, 512], F32, tag="re", name="angc", bufs=1)
            nc.vector.tensor_scalar(
                out=angc, in0=ps_a, scalar1=float(np.pi / 2), scalar2=None,
                op0=mybir.AluOpType.add,
            )
            reduced_sin(cs4[:, sl], angc)
        # bake the rotate-half signs: rows [0:32] and [64:96] get -sin
        nc.vector.tensor_scalar(
            out=sns4[0:32, :], in0=sns4[0:32, :], scalar1=-1.0, scalar2=None,
            op0=mybir.AluOpType.mult,
        )
        nc.vector.tensor_scalar(
            out=sns4[64:96, :], in0=sns4[64:96, :], scalar1=-1.0, scalar2=None,
            op0=mybir.AluOpType.mult,
        )

        # ---- QKV projections (+ RoPE rotation for Q, K) ----
        def project_rot(w_sb, dst, ch):
            # one 1024-token chunk: psum = (W_c^T)^T-style accumulation
            ps = ps_main.tile([128, 1024], F32, tag="big")
            for k in range(KS):
                for hh in range(2):
                    nc.tensor.matmul(
                        ps[:, hh * 512:(hh + 1) * 512],
                        w_sb[:, k, :],
                        xt[:, k, ch * 1024 + hh * 512: ch * 1024 + (hh + 1) * 512],
                        start=(k == 0),
                        stop=(k == KS - 1),
                    )
            # RoPE (rotate-half): rows h*64+[0:32] = x1 (even dims),
            # h*64+[32:64] = x2 (odd dims).  rot = x*cos + swap(x)*sns where
            # swap exchanges the two 32-row halves of each head (via DMA,
            # since DVE lanes cannot cross partitions).
            ts = (ch % (T // 1024)) * 1024  # position within batch
            t1 = tmp.tile([128, 1024], BF16, tag="t1")
            nc.vector.tensor_mul(t1, ps, cs4[:, ts:ts + 1024])
            xc = tmp.tile([128, 1024], BF16, tag="xc")
            nc.scalar.copy(out=xc, in_=ps)
            xs = tmp.tile([128, 1024], BF16, tag="xs")
            for h in range(HPC):
                o = h * 64
                nc.sync.dma_start(out=xs[o:o + 32, :], in_=xc[o + 32:o + 64, :])
                nc.sync.dma_start(out=xs[o + 32:o + 64, :], in_=xc[o:o + 32, :])
            t2 = tmp.tile([128, 1024], BF16, tag="t2")
            nc.gpsimd.tensor_mul(t2, xs, sns4[:, ts:ts + 1024])
            cols = slice(ch * 1024, (ch + 1) * 1024)
            nc.vector.tensor_add(dst[:, cols], t1, t2)

        for ch in range(TOK // 1024):
            project_rot(wq, qt, ch)
            project_rot(wk, kt, ch)

        # ---- attention (per batch, per 1024-token group) ----
        a2a_in = dpool.tile([NCORES, 128, TPC], BF16, tag="a2a_in")
        a2a_out = dpool.tile([NCORES, 128, TPC], BF16, tag="a2a_out")

        # V production is interleaved into the attention loop as filler PE
        # work: each produce_v is ~8 dense matmuls with no dependency on
        # exp, lengthening per-keytile PE bursts past the ~3.4us HAM
        # warm-up threshold so attention matmuls run at full clock.
        vjobs = [(b_, kt_) for b_ in range(B) for kt_ in range(NKT)]

        def produce_v(bv, ktv):
            ps_v = ps_main.tile([128, 1024], F32, tag="big", name="ps_v")
            t0 = bv * T + ktv * 128
            for k in range(KS):
                nc.tensor.matmul(
                    ps_v[:, 0:128],
                    xt[:, k, t0:t0 + 128],
                    wv[:, k, :],
                    start=(k == 0),
                    stop=(k == KS - 1),
                )
            for h in range(HPC):
                nc.vector.tensor_copy(
                    out=vsb[:, bv, ktv, 65 * h:65 * h + 64],
                    in_=ps_v[:, h * 64:(h + 1) * 64],
                )

        for b in range(B):
            for c2 in range(T // 1024):
                # ensure every V keytile this group consumes is produced
                while vjobs and (vjobs[0][0] < b or (
                        vjobs[0][0] == b and vjobs[0][1] < 8 * (c2 + 1))):
                    produce_v(*vjobs.pop(0))
                # live PV accumulators: [65, 512] per (head, chunk-half)
                pv = {}
                for h in range(HPC):
                    for cl in range(2):
                        pv[(h, cl)] = ps_pv.tile(
                            [65, 512], F32, tag="pv", name=f"pv{h}{cl}"
                        )
                # software-pipelined: emit PV matmuls of keytile k-1 after
                # the scores+exp of keytile k, so PE's in-order stream never
                # stalls on the exp/mask it is itself supposed to overlap
                pv_work = []

                def emit_epilogue(h, cl):
                    # normalize + ship this chunk as soon as its PV stops,
                    # overlapping remaining keytiles of the group
                    c = 2 * c2 + cl
                    rcp = tmp.tile([65, 512], F32, tag="rcp", name="rcp")
                    nc.vector.reciprocal(rcp[64:65, :], pv[(h, cl)][64:65, :])
                    bc_ps = ps_main.tile([64, 512], F32, tag="big", name="bc_ps")
                    nc.tensor.matmul(
                        bc_ps, ones65[64:65, :], rcp[64:65, :],
                        start=True, stop=True,
                    )
                    bc = tmp.tile([64, 512], F32, tag="bc", name="bc")
                    nc.vector.tensor_copy(out=bc, in_=bc_ps)
                    att = apool.tile([64, 512], BF16, tag="att", name="att")
                    nc.vector.tensor_mul(att, pv[(h, cl)][0:64, :], bc)
                    nc.sync.dma_start(
                        out=a2a_in[b * NCH + c, h * 64:(h + 1) * 64, :], in_=att
                    )

                def emit_pv(work):
                    h, ktile, p_sb = work
                    for cl in range(2):
                        c = 2 * c2 + cl  # global 512-chunk in batch
                        if ktile > 4 * c + 3:
                            continue
                        nc.tensor.matmul(
                            pv[(h, cl)],
                            vsb[:, b, ktile, 65 * h:65 * h + 65],
                            p_sb[:, cl * 512:(cl + 1) * 512],
                            start=(ktile == 0),
                            stop=(ktile == 4 * c + 3),
                        )
                        if ktile == 4 * c + 3:
                            emit_epilogue(h, cl)

                for ktile in range(8 * (c2 + 1)):
                    if vjobs:  # filler: keep the PE dense between exp waits
                        produce_v(*vjobs.pop(0))
                    ts0 = max(0, ktile * 128 - c2 * 1024)  # first valid tok
                    for h in range(HPC):
                        o = h * 64
                        ps_s = ps_main.tile([128, 1024], F32, tag="big")
                        for hh in range(ts0 // 512, 2):
                            nc.tensor.matmul(
                                ps_s[:, hh * 512:(hh + 1) * 512],
                                kt[o:o + 64, b * T + ktile * 128: b * T + ktile * 128 + 128],
                                qt[o:o + 64, b * T + c2 * 1024 + hh * 512: b * T + c2 * 1024 + (hh + 1) * 512],
                                start=True,
                                stop=True,
                            )
                        p_sb = ppool.tile([128, 1024], BF16, tag="p")
                        nc.scalar.activation(
                            out=p_sb[:, ts0:1024], in_=ps_s[:, ts0:1024],
                            func=mybir.ActivationFunctionType.Exp, scale=SCALE,
                        )
                        # mask the diagonal block (keys > tok -> 0); only when
                        # this keytile's diagonal falls inside this tok window
                        if ktile * 128 >= c2 * 1024:
                            nc.vector.tensor_mul(
                                p_sb[:, ts0:ts0 + 128], p_sb[:, ts0:ts0 + 128], tri_sb
                            )
                        # zero the pre-diagonal gap inside the first chunk-half
                        g0 = (ts0 // 512) * 512
                        if ts0 > g0:
                            nc.gpsimd.memset(p_sb[:, g0:ts0], 0.0)
                        pv_work.append((h, ktile, p_sb))
                        if len(pv_work) > 2:
                            emit_pv(pv_work.pop(0))
                for w in pv_work:
                    emit_pv(w)

        # ---- AllToAll: head-sharded -> token-sharded ----
        if skip_collective:
            # timeline-model variant: stand-in DRAM copy instead of the
            # collective (TimelineSim cannot model collectives)
            nc.sync.dma_start(out=a2a_out[:], in_=a2a_in[:])
        else:
            nc.gpsimd.collective_compute(
                "AllToAll",
                mybir.AluOpType.bypass,
                replica_groups=[list(range(NCORES))],
                ins=[a2a_in[:]],
                outs=[a2a_out[:]],
            )

        # ---- output projection for this core's 512 tokens ----
        ao = singles.tile([128, KS, TPC], BF16, tag="ao")
        for s in range(KS):
            nc.sync.dma_start(out=ao[:, s, :], in_=a2a_out[s, :, :])
        for m in range(KS):
            ps_o = ps_main.tile([128, 1024], F32, tag="big")
            for s in range(KS):
                nc.tensor.matmul(
                    ps_o[:, 0:TPC],
                    wo[:, s, m * 128:(m + 1) * 128],
                    ao[:, s, :],
                    start=(s == 0),
                    stop=(s == KS - 1),
                )
            osb = tmp.tile([128, TPC], F32, tag="osb")
            nc.vector.tensor_copy(out=osb, in_=ps_o[:, 0:TPC])
            nc.sync.dma_start(out=out_d[m * 128:(m + 1) * 128, :], in_=osb)

    nc.compile()
    return nc


_NC = None


def _get_nc():
    global _NC
    if _NC is None:
        _NC = build_nc()
    return _NC


def _rope_perm():
    # per-head deinterleave: evens first then odds, applied to a 128-row slice
    p = []
    for h in range(HPC):
        p.extend(h * DH + np.arange(0, DH, 2))
        p.extend(h * DH + np.arange(1, DH, 2))
    return np.asarray(p)


def make_in_maps(inputs):
    x = np.asarray(inputs["in_features"], np.float32)
    Wq = np.asarray(inputs["Wq"], np.float32)
    Wk = np.asarray(inputs["Wk"], np.float32)
    Wv = np.asarray(inputs["Wv"], np.float32)
    Wo = np.asarray(inputs["Wo"], np.float32)
    pos = np.asarray(inputs["token_positions"]).astype(np.float32)[None, :]

    bf = ml_dtypes.bfloat16
    XT = np.ascontiguousarray(x.reshape(TOK, D).T).astype(bf)
    WoT = np.ascontiguousarray(Wo.T).astype(bf)
    perm = _rope_perm()

    in_maps = []
    for c in range(NCORES):
        rows = slice(c * 128, (c + 1) * 128)
        wq_c = Wq[rows][perm]
        wk_c = Wk[rows][perm]
        wv_c = Wv[rows]
        in_maps.append({
            "xt": XT,
            "wqt": np.ascontiguousarray(wq_c.T).astype(bf),
            "wkt": np.ascontiguousarray(wk_c.T).astype(bf),
            "wvt": np.ascontiguousarray(wv_c.T).astype(bf),
            "wot": WoT,
            "posf": pos,
        })
    return in_maps


def assemble_out(results):
    full = np.empty((TOK, D), np.float32)
    for c in range(NCORES):
        chunk = np.asarray(results[c]["out"])  # [D, 512] = out^T slice
        t0 = (c // 4) * T + (c % 4) * TPC
        full[t0:t0 + TPC] = chunk.T
    return full.reshape(B, T, D)


def run(inputs, **kwargs):
    nc = _get_nc()
    res = run_bass_kernel_spmd(
        nc, make_in_maps(inputs), core_ids=list(range(NCORES)), **kwargs
    )
    return assemble_out(res.results), res


def kernel(**inputs) -> np.ndarray:
    out, _ = run(inputs)
    return out


# ---------------------------------------------------------------------------
# Benchmark path: cached jitted executable so repeat executions can be timed
# without retracing/recompiling. Mirrors bass2jax.run_bass_via_pjrt.
# ---------------------------------------------------------------------------
_EXEC = {}


def _build_exec(kind="main"):
    if kind in _EXEC:
        return _EXEC[kind]
    import jax
    from jax.experimental.shard_map import shard_map
    from jax.sharding import Mesh, PartitionSpec

    import concourse.mybir as mybir
    from concourse import bass2jax

    nc = _get_nc() if kind == "main" else build_nc(dummy=True)
    bass2jax.install_neuronx_cc_hook()

    partition_name = nc.partition_id_tensor.name if nc.partition_id_tensor else None
    in_names, out_names, out_avals, zero_outs = [], [], [], []
    for alloc in nc.m.functions[0].allocations:
        if not isinstance(alloc, mybir.MemoryLocationSet):
            continue
        name = alloc.memorylocations[0].name
        if alloc.kind == "ExternalInput":
            if name != partition_name:
                in_names.append(name)
        elif alloc.kind == "ExternalOutput":
            out_names.append(name)
            shape = tuple(alloc.tensor_shape)
            dtype = mybir.dt.np(alloc.dtype)
            out_avals.append(jax.core.ShapedArray(shape, dtype))
            zero_outs.append(np.zeros(shape, dtype))
    n_params = len(in_names)
    all_names = list(in_names) + list(out_names)
    if partition_name is not None:
        all_names.append(partition_name)

    def _body(*args):
        outs = bass2jax._bass_exec_p.bind(
            *(list(args) + ([bass2jax.partition_id_tensor()]
                            if partition_name is not None else [])),
            out_avals=tuple(out_avals),
            in_names=tuple(all_names),
            out_names=tuple(out_names),
            lowering_input_output_aliases=(),
            sim_require_finite=True,
            sim_require_nnan=True,
            nc=nc,
        )
        return tuple(outs)

    devices = jax.devices()[:NCORES]
    mesh = Mesh(np.asarray(devices), ("core",))
    nspec = n_params + len(out_names)
    sharded = jax.jit(
        shard_map(
            _body,
            mesh=mesh,
            in_specs=(PartitionSpec("core"),) * nspec,
            out_specs=(PartitionSpec("core"),) * len(out_names),
            check_rep=False,
        ),
        keep_unused=True,
    )
    _EXEC[kind] = (sharded, in_names, out_names, zero_outs, mesh)
    return _EXEC[kind]


def _stage_args(inputs, exec_tuple):
    import jax
    from jax.sharding import NamedSharding, PartitionSpec

    sharded, in_names, out_names, zero_outs, mesh = exec_tuple
    in_maps = make_in_maps(inputs)
    sh = NamedSharding(mesh, PartitionSpec("core"))
    args = []
    for name in in_names:
        cat = np.concatenate([in_maps[c][name] for c in range(NCORES)], axis=0)
        args.append(jax.device_put(cat, sh))
    for z in zero_outs:
        cat = np.concatenate([z] * NCORES, axis=0)
        args.append(jax.device_put(cat, sh))
    return args


def _timed(fn, args, iters):
    import time

    import jax

    jax.block_until_ready(fn(*args))
    best = float("inf")
    for _ in range(iters):
        t0 = time.perf_counter()
        outs = fn(*args)
        jax.block_until_ready(outs)
        best = min(best, time.perf_counter() - t0)
    return outs, best


def run_bench(inputs, iters=10):
    """Returns (output, est_exec_seconds, t_full, t_dummy): times the real
    kernel and a near-empty NEFF with identical I/O; the difference cancels
    the axon dispatch + NEFF-launch overhead."""
    e1 = _build_exec("main")
    args = _stage_args(inputs, e1)
    outs, t_full = _timed(e1[0], args, iters)
    ed = _build_exec("dummy")
    _timed(ed[0], args, iters)
    _, t_dummy = _timed(ed[0], args, iters)

    _, in_names, out_names, zero_outs, mesh = e1
    results = []
    for c in range(NCORES):
        m = {}
        for i, name in enumerate(out_names):
            arr = np.asarray(outs[i])
            per = arr.shape[0] // NCORES
            m[name] = arr[c * per:(c + 1) * per]
        results.append(m)
    return assemble_out(results), max(t_full - t_dummy, 0.0), t_full, t_dummy




# revision 18
# speedup vs baseline: 1.4514x; 1.4514x over previous
"""Trainium2 Bass kernel for RoPE multi-head attention (B=2, T=2048, D=1024, H=16).

Sharding: tensor-parallel over heads (2 heads / core). Single dense PE
stream: QKV projections, V production and the output projection are woven
as filler between the exp-dependent attention matmuls so the PE never
idles long enough for the HAM clock gate to re-throttle it to 1.2 GHz.

  - Scores computed transposed (S^T = K @ Q^T) per 128-key tile over a
    1024-token window, restricted to the causal column range; exp on
    ScalarE (its only job); the diagonal 128x128 block is masked with a
    0/1 triangle after exp (DVE).
  - PV accumulates in PSUM per (head, 512-tok chunk) with a ones column
    in V producing the softmax denominator for free. Matmul column
    ranges are causal-restricted; PSUM has_written handles the gaps.
  - Normalization is deferred past the collective: raw PV + rowsums ship
    through the AllToAll; the destination core computes 1/rowsum with
    reciprocal_approx_fast and broadcasts it across partitions with a
    K=2 selector matmul (no 3.4us single-lane RECIPROCALs, no per-
    epilogue PE broadcast on the critical path).
  - Token remap: core c owns tokens [256c, 256c+256) of EACH batch, so
    the AllToAll splits in two: batch-0's collective + its half of the
    output projection overlap batch-1's attention; only batch-1's small
    collective is exposed in the tail.
  - X^T DMA is issued in 512-token-chunk-major order so the first
    projection can start after ~2 MiB instead of the full 8 MiB.
Host side only reshapes / transposes / casts / shards.
"""

import sys
from contextlib import ExitStack

for _p in ("/opt/trn_rl_repo",):
    if _p not in sys.path:
        sys.path.append(_p)

import numpy as np
import ml_dtypes

import concourse.bacc as bacc
import concourse.bass as bass
import concourse.mybir as mybir
import concourse.tile as tile
from concourse.bass_utils import run_bass_kernel_spmd

# Problem constants (hardcoded per harness contract).
B, T, D, H, DH = 2, 2048, 1024, 16, 64
NCORES = 8
HPC = H // NCORES          # heads per core = 2
TOK = B * T                # 4096 flattened tokens
THETA = 10000.0
SCALE = 1.0 / 8.0          # 1/sqrt(DH)
KS = D // 128              # 8 contraction slices
NKT = T // 128             # 16 key tiles per batch
TPC = 512                  # output tokens per core (256 per batch)
LAG = 2                    # scores->PV software pipeline depth (tiles)

BF16 = mybir.dt.bfloat16
F32 = mybir.dt.float32


def build_nc(skip_collective=False, dummy=False, dbg=False):
    nc = bacc.Bacc(
        "TRN2",
        target_bir_lowering=False,
        debug=False,
        num_devices=NCORES,
    )
    dbg_d = {}
    if dbg:
        dbg_d["qt"] = nc.dram_tensor("dbg_qt", [128, TOK], BF16, kind="ExternalOutput")
        dbg_d["kt"] = nc.dram_tensor("dbg_kt", [128, TOK], BF16, kind="ExternalOutput")
        dbg_d["a2a"] = nc.dram_tensor(
            "dbg_a2a", [B * NCORES, 130, 256], BF16, kind="ExternalOutput")
        dbg_d["rs"] = nc.dram_tensor("dbg_rs", [16, 512], F32, kind="ExternalOutput")
        dbg_d["rcp"] = nc.dram_tensor("dbg_rcp", [16, 512], F32, kind="ExternalOutput")

    # ---- kernel I/O ----
    xt_d = nc.dram_tensor("xt", [D, TOK], BF16, kind="ExternalInput")
    wqt_d = nc.dram_tensor("wqt", [D, 128], BF16, kind="ExternalInput")
    wkt_d = nc.dram_tensor("wkt", [D, 128], BF16, kind="ExternalInput")
    wvt_d = nc.dram_tensor("wvt", [D, 128], BF16, kind="ExternalInput")
    wot_d = nc.dram_tensor("wot", [D, D], BF16, kind="ExternalInput")
    posf_d = nc.dram_tensor("posf", [1, T], F32, kind="ExternalInput")
    out_d = nc.dram_tensor("out", [D, TPC], F32, kind="ExternalOutput")

    # ---- compile-time constants ----
    inv_freq = (1.0 / THETA ** (np.arange(DH // 2, dtype=np.float64) / (DH // 2)))
    invf4 = np.tile(inv_freq.astype(np.float32), 4)[None, :]  # [1, 128]
    invf_d = nc.inline_tensor(invf4, "invf")
    # tri[j, i] = 1 iff key j <= tok i (keeps lower-triangular attention).
    tri = np.triu(np.ones((128, 128), np.float32)).astype(ml_dtypes.bfloat16)
    tri_d = nc.inline_tensor(tri, "tri")
    # rs layout: row s = head 2s (g=0), row 8+s = head 2s+1 (g=1) so each
    # gather DMA covers a contiguous partition range. sel16[r, s, p] = 1 iff
    # r == 8*(p//64) + s: slice s broadcasts its two heads' reciprocal rows
    # across the two 64-partition blocks via a K=16 matmul.
    sel16 = np.zeros((16, KS, 128), np.float32)
    for s in range(KS):
        sel16[s, s, 0:64] = 1.0
        sel16[8 + s, s, 64:128] = 1.0
    sel16_d = nc.inline_tensor(
        sel16.reshape(16, KS * 128).astype(ml_dtypes.bfloat16), "sel16")

    if dummy:
        # identical I/O signature, near-empty body: used to measure the
        # fixed dispatch/NEFF-launch overhead so it can be subtracted
        with tile.TileContext(nc) as tc, ExitStack() as ctx:
            sp = ctx.enter_context(tc.tile_pool(name="sp", bufs=1))
            z = sp.tile([128, 16], F32, tag="z")
            nc.vector.memset(z, 0.0)
            nc.sync.dma_start(out=out_d[0:128, 0:16], in_=z)
        nc.compile()
        return nc

    with tile.TileContext(nc) as tc, ExitStack() as ctx:
        singles = ctx.enter_context(tc.tile_pool(name="singles", bufs=1))
        tmp = ctx.enter_context(tc.tile_pool(name="tmp", bufs=2))
        rtmp = ctx.enter_context(tc.tile_pool(name="rtmp", bufs=1))
        ppool = ctx.enter_context(tc.tile_pool(name="ppool", bufs=6))
        apool = ctx.enter_context(tc.tile_pool(name="apool", bufs=4))
        aop = ctx.enter_context(tc.tile_pool(name="aop", bufs=1))
        ps_sc = ctx.enter_context(tc.tile_pool(name="ps_sc", bufs=2, space="PSUM"))
        ps_pv = ctx.enter_context(tc.tile_pool(name="ps_pv", bufs=2, space="PSUM"))
        ps_sm = ctx.enter_context(tc.tile_pool(name="ps_sm", bufs=1, space="PSUM"))
        ps_pj = ctx.enter_context(tc.tile_pool(name="ps_pj", bufs=1, space="PSUM"))
        dpool = ctx.enter_context(tc.tile_pool(name="dram", bufs=1, space="DRAM"))

        # ---- persistent SBUF tensors; small inputs first ----
        posf = singles.tile([1, T], F32, tag="posf")
        nc.sync.dma_start(out=posf, in_=posf_d.ap())
        invf = singles.tile([1, 128], F32, tag="invf")
        nc.sync.dma_start(out=invf, in_=invf_d.ap())
        tri_sb = singles.tile([128, 128], BF16, tag="tri")
        nc.sync.dma_start(out=tri_sb, in_=tri_d.ap())
        sel16_sb = singles.tile([16, KS, 128], BF16, tag="sel16")
        nc.sync.dma_start(
            out=sel16_sb, in_=sel16_d.ap().rearrange("h (s p) -> h s p", p=128))
        # warmup collective: absorbs the one-time ~26us comm-setup barrier
        # while the input DMAs / projections run.
        warm_i = dpool.tile([1, 4], F32, tag="warm_i", name="warm_i")
        warm_o = dpool.tile([1, 4], F32, tag="warm_o", name="warm_o")
        wz = singles.tile([1, 4], F32, tag="wz")
        nc.vector.memset(wz, 0.0)
        nc.sync.dma_start(out=warm_i[:], in_=wz)
        if not skip_collective:
            nc.gpsimd.collective_compute(
                "AllReduce",
                mybir.AluOpType.add,
                replica_groups=[list(range(NCORES))],
                ins=[warm_i[:]],
                outs=[warm_o[:]],
            )
        wq = singles.tile([128, KS, 128], BF16, tag="wq")
        nc.sync.dma_start(out=wq, in_=wqt_d.ap().rearrange("(k p) m -> p k m", p=128))
        wk = singles.tile([128, KS, 128], BF16, tag="wk")
        nc.sync.dma_start(out=wk, in_=wkt_d.ap().rearrange("(k p) m -> p k m", p=128))
        wv = singles.tile([128, KS, 128], BF16, tag="wv")
        nc.sync.dma_start(out=wv, in_=wvt_d.ap().rearrange("(k p) m -> p k m", p=128))
        # X^T loaded chunk-major: all 8 k-slices of 512 tokens per DMA so
        # the first projection can start as soon as chunk 0 lands.
        xt = singles.tile([128, KS, TOK], BF16, tag="xt")
        xt_r = xt_d.ap().rearrange("(k p) t -> p k t", p=128)
        for ch in range(TOK // 512):
            sl = slice(ch * 512, (ch + 1) * 512)
            nc.sync.dma_start(out=xt[:, :, sl], in_=xt_r[:, :, sl])
        wo = singles.tile([128, KS, D], BF16, tag="wo")
        wo_r = wot_d.ap().rearrange("(k p) f -> p k f", p=128)
        for k in range(KS):
            nc.sync.dma_start(out=wo[:, k, :], in_=wo_r[:, k, :])

        qt = singles.tile([128, TOK], BF16, tag="qt")     # rotated Q^T
        kt = singles.tile([128, TOK], BF16, tag="kt")     # rotated K^T
        # V per (batch, keytile): [keys=128, 65*HPC]; col 64/129 = ones
        # (softmax denominator accumulates on pv partition 64).
        vsb = singles.tile([128, B, NKT, 65 * HPC], BF16, tag="vsb")
        nc.vector.memset(vsb[:, :, :, 64:65], 1.0)
        nc.vector.memset(vsb[:, :, :, 129:130], 1.0)

        # cos table replicated over 4 row-blocks; sin table with sign baked
        # per half: rows [0:32] -sin, [32:64] +sin, then repeated.
        cs4 = singles.tile([128, T], F32, tag="cs4")
        sns4 = singles.tile([128, T], F32, tag="sns4")

        # ---- RoPE tables: ang = pos * inv_freq, sin/cos on ScalarE ----
        # ScalarE Sin needs args in [-pi, pi]. Range-reduce with the fp32
        # magic-number round: rn(x) = (x + 1.5*2^23) - 1.5*2^23, so
        # red = ang - 2pi*rn(ang/2pi) lands in [-pi, pi] (clamped for safety).
        tau = float(2 * np.pi)
        magic = float(1.5 * 2 ** 23)

        def reduced_sin(dst, src_ap):
            qm = rtmp.tile([128, 512], F32, tag="ra", name="qm")
            nc.vector.tensor_scalar(
                out=qm, in0=src_ap, scalar1=1.0 / tau, scalar2=magic,
                op0=mybir.AluOpType.mult, op1=mybir.AluOpType.add,
            )
            qr = rtmp.tile([128, 512], F32, tag="rb", name="qr")
            nc.vector.tensor_scalar(
                out=qr, in0=qm, scalar1=magic, scalar2=None,
                op0=mybir.AluOpType.subtract,
            )
            red = rtmp.tile([128, 512], F32, tag="rc", name="red")
            nc.vector.scalar_tensor_tensor(
                out=red, in0=qr, scalar=-tau, in1=src_ap,
                op0=mybir.AluOpType.mult, op1=mybir.AluOpType.add,
            )
            redc = rtmp.tile([128, 512], F32, tag="rd", name="redc")
            nc.vector.tensor_scalar(
                out=redc, in0=red, scalar1=float(np.pi), scalar2=float(-np.pi),
                op0=mybir.AluOpType.min, op1=mybir.AluOpType.max,
            )
            nc.scalar.activation(
                out=dst, in_=redc, func=mybir.ActivationFunctionType.Sin
            )

        for q4 in range(T // 512):
            pool = ps_sm if q4 % 2 == 0 else ps_pj
            ps_a = pool.tile([128, 512], F32, tag="sm" if q4 % 2 == 0 else "pj",
                             name=f"ang{q4}")
            nc.tensor.matmul(
                ps_a, invf, posf[:, q4 * 512:(q4 + 1) * 512], start=True, stop=True
            )
            sl = slice(q4 * 512, (q4 + 1) * 512)
            reduced_sin(sns4[:, sl], ps_a)
            angc = rtmp.tile([128, 512], F32, tag="re", name="angc")
            nc.vector.tensor_scalar(
                out=angc, in0=ps_a, scalar1=float(np.pi / 2), scalar2=None,
                op0=mybir.AluOpType.add,
            )
            reduced_sin(cs4[:, sl], angc)
        # bake the rotate-half signs: rows [0:32] and [64:96] get -sin
        nc.vector.tensor_scalar(
            out=sns4[0:32, :], in0=sns4[0:32, :], scalar1=-1.0, scalar2=None,
            op0=mybir.AluOpType.mult,
        )
        nc.vector.tensor_scalar(
            out=sns4[64:96, :], in0=sns4[64:96, :], scalar1=-1.0, scalar2=None,
            op0=mybir.AluOpType.mult,
        )

        # ---- QKV projection (+ RoPE rotation) for one 512-token chunk ----
        # rows h*64+[0:32] = x1 (even dims), h*64+[32:64] = x2 (odd dims)
        # via the host-side row permutation. rot = x*cos + swap(x)*sns where
        # swap exchanges the two 32-row halves of each head (via DMA,
        # since DVE lanes cannot cross partitions).
        def project_rot(w_sb, dst, ch, t2_dve=False):
            ps = ps_pj.tile([128, 512], F32, tag="pj", name=f"pj_{ch}")
            sl = slice(ch * 512, (ch + 1) * 512)
            for k in range(KS):
                nc.tensor.matmul(
                    ps, w_sb[:, k, :], xt[:, k, sl],
                    start=(k == 0), stop=(k == KS - 1),
                )
            ts = (ch % (T // 512)) * 512  # position within batch
            t1 = tmp.tile([128, 512], BF16, tag="t1", name="t1")
            nc.vector.tensor_mul(t1, ps, cs4[:, ts:ts + 512])
            xc = tmp.tile([128, 512], BF16, tag="xc", name="xc")
            nc.vector.tensor_copy(out=xc, in_=ps)
            xs = tmp.tile([128, 512], BF16, tag="xs", name="xs")
            for h in range(HPC):
                o = h * 64
                nc.sync.dma_start(out=xs[o:o + 32, :], in_=xc[o + 32:o + 64, :])
                nc.sync.dma_start(out=xs[o + 32:o + 64, :], in_=xc[o:o + 32, :])
            t2 = tmp.tile([128, 512], BF16, tag="t2", name="t2")
            eng = nc.vector if t2_dve else nc.gpsimd
            eng.tensor_mul(t2, xs, sns4[:, ts:ts + 512])
            nc.vector.tensor_add(dst[:, sl], t1, t2)

        # ---- V production: one 128-token key tile ----
        def produce_v(bv, ktv):
            ps_v = ps_sm.tile([128, 512], F32, tag="sm", name=f"v{bv}_{ktv}")
            t0 = bv * T + ktv * 128
            for k in range(KS):
                nc.tensor.matmul(
                    ps_v[:, 0:128],
                    xt[:, k, t0:t0 + 128],
                    wv[:, k, :],
                    start=(k == 0),
                    stop=(k == KS - 1),
                )
            for h in range(HPC):
                nc.vector.tensor_copy(
                    out=vsb[:, bv, ktv, 65 * h:65 * h + 64],
                    in_=ps_v[:, h * 64:(h + 1) * 64],
                )

        # ---- a2a staging: one collective per batch ----
        # slot j of batch b = this core's head-dims + rowsums for dest core
        # j's 256 tokens [256j, 256j+256) of batch b. Rows: 0:64 = h0 dims,
        # 64 = h0 rowsum, 65:129 = h1 dims, 129 = h1 rowsum.
        a2a_in = [dpool.tile([NCORES, 130, 256], BF16, tag=f"a2ai{b}",
                             name=f"a2ai{b}") for b in range(B)]
        a2a_out = [dpool.tile([NCORES, 130, 256], BF16, tag=f"a2ao{b}",
                              name=f"a2ao{b}") for b in range(B)]

        b0_done = [0]

        def epilogue(b, c2, h, cl, pvt):
            att = apool.tile([65, 512], BF16, tag="att", name=f"att{b}{c2}{h}{cl}")
            nc.vector.tensor_copy(out=att, in_=pvt)
            j0 = 2 * (2 * c2 + cl)
            for jj in range(2):
                nc.sync.dma_start(
                    out=a2a_in[b][j0 + jj, 65 * h:65 * h + 65, :],
                    in_=att[:, jj * 256:(jj + 1) * 256],
                )
            if b == 0:
                b0_done[0] += 1
                if b0_done[0] == 8:
                    a2a(0)

        pv_ctx = {}
        pvq = []

        def emit_pv(b, c2, h, kt_i, p_sb):
            key = (b, c2, h)
            if key not in pv_ctx:
                pv_ctx[key] = {
                    cl: ps_pv.tile([65, 512], F32, tag="pv",
                                   name=f"pv{b}{c2}{h}{cl}")
                    for cl in range(2)
                }
            pv = pv_ctx[key]
            ts0 = max(0, kt_i * 128 - c2 * 1024)
            for cl in range(2):
                c = 2 * c2 + cl
                if kt_i > 4 * c + 3:
                    continue
                lo = max(0, ts0 - 512 * cl)
                nc.tensor.matmul(
                    pv[cl][:, lo:512],
                    vsb[:, b, kt_i, 65 * h:65 * h + 65],
                    p_sb[:, 512 * cl + lo:512 * (cl + 1)],
                    start=(kt_i == 0),
                    stop=(kt_i == 4 * c + 3),
                )
                if kt_i == 4 * c + 3:
                    epilogue(b, c2, h, cl, pv[cl])

        def drain(n):
            while len(pvq) > n:
                emit_pv(*pvq.pop(0))

        # ---- attention over one (batch, 1024-token group) ----
        # One continuous scores->exp->PV pipeline across every (batch,
        # group, head): no per-head flush, so the PE never drains while
        # ScalarE's exp backlog catches up.
        def attention(b, c2, fillers):
            nkt_g = 8 * (c2 + 1)
            for h in range(HPC):
                o = h * 64
                for kt_i in range(nkt_g):
                    for thunk in fillers.get((h, kt_i), ()):
                        thunk()
                    ts0 = max(0, kt_i * 128 - c2 * 1024)
                    ps_s = ps_sc.tile([128, 1024], F32, tag="sc",
                                      name=f"sc{b}{c2}{h}{kt_i}")
                    kcol = b * T + kt_i * 128
                    qbase = b * T + c2 * 1024
                    for hh in range(2):
                        lo = max(0, ts0 - 512 * hh)
                        if lo >= 512:
                            continue
                        nc.tensor.matmul(
                            ps_s[:, 512 * hh + lo:512 * (hh + 1)],
                            kt[o:o + 64, kcol:kcol + 128],
                            qt[o:o + 64, qbase + 512 * hh + lo:qbase + 512 * (hh + 1)],
                            start=True,
                            stop=True,
                        )
                    p_sb = ppool.tile([128, 1024], BF16, tag="p", name="p_sb")
                    nc.scalar.activation(
                        out=p_sb[:, ts0:1024], in_=ps_s[:, ts0:1024],
                        func=mybir.ActivationFunctionType.Exp, scale=SCALE,
                    )
                    # mask the diagonal block (keys > tok -> 0)
                    if kt_i * 128 >= c2 * 1024:
                        nc.vector.tensor_mul(
                            p_sb[:, ts0:ts0 + 128], p_sb[:, ts0:ts0 + 128], tri_sb
                        )
                    pvq.append((b, c2, h, kt_i, p_sb))
                    drain(LAG)

        # ---- normalization + output projection for one batch half ----
        out_r = out_d.ap().rearrange("(m p) t -> p m t", p=128)
        aos = {}
        osbs = {}

        def normalize(half):
            src = a2a_out[half]
            ao = aop.tile([128, KS, 256], BF16, tag=f"ao{half}", name=f"ao{half}")
            aos[half] = ao
            osbs[half] = singles.tile([128, KS, 256], F32, tag=f"osb{half}",
                                      name=f"osb{half}")
            for g in range(2):
                nc.sync.dma_start(
                    out=ao[64 * g:64 * g + 64, :, :],
                    in_=src[:, 65 * g:65 * g + 64, :].rearrange("s r c -> r s c"),
                )
            rs = tmp.tile([16, 256], BF16, tag="rs", name="rs")
            nc.sync.dma_start(out=rs[0:8, :], in_=src[:, 64, :])
            nc.sync.dma_start(out=rs[8:16, :], in_=src[:, 129, :])
            rsf = tmp.tile([16, 256], F32, tag="rsf", name="rsf")
            nc.vector.tensor_copy(out=rsf, in_=rs)
            rcb = tmp.tile([16, 256], F32, tag="rcb", name="rcb")
            nc.vector.reciprocal_approx_fast(out=rcb, in_=rsf)
            rcp16 = tmp.tile([16, 256], BF16, tag="rcp16", name="rcp16")
            nc.vector.tensor_copy(out=rcp16, in_=rcb)
            if dbg:
                sl = slice(half * 256, (half + 1) * 256)
                nc.sync.dma_start(out=dbg_d["rs"][:, sl], in_=rsf)
                nc.sync.dma_start(out=dbg_d["rcp"][:, sl], in_=rcb)
                nc.sync.dma_start(
                    out=dbg_d["a2a"][half * NCORES:(half + 1) * NCORES], in_=src[:])
            for s in range(KS):
                pool, tg = (ps_sm, "sm") if s % 2 == 0 else (ps_pj, "pj")
                mult = pool.tile([128, 512], F32, tag=tg, name=f"mult{half}{s}")
                nc.tensor.matmul(
                    mult[:, 0:256], sel16_sb[:, s, :], rcp16,
                    start=True, stop=True,
                )
                nc.vector.tensor_mul(ao[:, s, :], ao[:, s, :], mult[:, 0:256])

        def oproj_m(half, m):
            ao = aos[half]
            pool, tg = (ps_sm, "sm") if m % 2 == 0 else (ps_pj, "pj")
            ps_o = pool.tile([128, 512], F32, tag=tg, name=f"o{half}{m}")
            for s in range(KS):
                nc.tensor.matmul(
                    ps_o[:, 0:256],
                    wo[:, s, m * 128:(m + 1) * 128],
                    ao[:, s, :],
                    start=(s == 0),
                    stop=(s == KS - 1),
                )
            nc.vector.tensor_copy(out=osbs[half][:, m, :], in_=ps_o[:, 0:256])
            if m == KS - 1:
                nc.sync.dma_start(
                    out=out_r[:, :, half * 256:(half + 1) * 256], in_=osbs[half])

        def a2a(b):
            if skip_collective:
                nc.sync.dma_start(out=a2a_out[b][:], in_=a2a_in[b][:])
            else:
                nc.gpsimd.collective_compute(
                    "AllToAll",
                    mybir.AluOpType.bypass,
                    replica_groups=[list(range(NCORES))],
                    ins=[a2a_in[b][:]],
                    outs=[a2a_out[b][:]],
                )

        # ---- emission schedule ----
        P = project_rot
        V = produce_v
        # phase 0: chunks 0,1 of q,k + V(b0, 0..7), staggered with the DMA
        P(wq, qt, 0)
        V(0, 0)
        V(0, 1)
        P(wk, kt, 0)
        V(0, 2)
        V(0, 3)
        P(wq, qt, 1)
        V(0, 4)
        V(0, 5)
        P(wk, kt, 1)
        V(0, 6)
        V(0, 7)

        attention(0, 0, {
            (0, 1): [lambda: P(wq, qt, 2)],
            (0, 3): [lambda: P(wk, kt, 2)],
            (0, 5): [lambda: V(0, 8)],
            (0, 6): [lambda: V(0, 9)],
            (0, 7): [lambda: V(0, 10)],
            (1, 1): [lambda: P(wq, qt, 3)],
            (1, 3): [lambda: P(wk, kt, 3)],
            (1, 4): [lambda: V(0, 11)],
            (1, 5): [lambda: V(0, 12)],
            (1, 6): [lambda: V(0, 13)],
            (1, 7): [lambda: V(0, 14), lambda: V(0, 15)],
        })
        attention(0, 1, {
            (0, 1): [lambda: P(wq, qt, 4)],
            (0, 4): [lambda: P(wk, kt, 4)],
            (0, 7): [lambda: V(1, 0)],
            (0, 9): [lambda: V(1, 1)],
            (0, 11): [lambda: V(1, 2)],
            (0, 13): [lambda: V(1, 3)],
            (1, 1): [lambda: P(wq, qt, 5)],
            (1, 4): [lambda: P(wk, kt, 5)],
            (1, 7): [lambda: V(1, 4)],
            (1, 9): [lambda: V(1, 5)],
            (1, 11): [lambda: V(1, 6)],
            (1, 13): [lambda: V(1, 7)],
        })
        a2a(0)
        # chunks 6,7 are emitted after the first collective trigger: keep
        # their RoPE mul off gpsimd in case the trigger parks its queue.
        attention(1, 0, {
            (0, 1): [lambda: P(wq, qt, 6, t2_dve=True)],
            (0, 4): [lambda: P(wk, kt, 6, t2_dve=True)],
            (0, 6): [lambda: V(1, 8)],
            (1, 1): [lambda: P(wq, qt, 7, t2_dve=True)],
            (1, 4): [lambda: P(wk, kt, 7, t2_dve=True)],
            (1, 6): [lambda: V(1, 9)],
        })
        attention(1, 1, {
            (0, 2): [lambda: V(1, 10)],
            (0, 4): [lambda: V(1, 11)],
            (0, 6): [lambda: V(1, 12)],
            (0, 8): [lambda: V(1, 13)],
            (0, 10): [lambda: V(1, 14)],
            (0, 12): [lambda: V(1, 15)],
            # batch-0 normalization + part of its output projection act as
            # PE filler while the exp backlog of the last tiles drains
            (1, 10): [lambda: normalize(0)],
            (1, 11): [lambda: oproj_m(0, 0)],
            (1, 12): [lambda: oproj_m(0, 1)],
            (1, 13): [lambda: oproj_m(0, 2)],
            (1, 14): [lambda: oproj_m(0, 3)],
            (1, 15): [lambda: oproj_m(0, 4)],
        })
        drain(0)
        a2a(1)
        if dbg:
            nc.sync.dma_start(out=dbg_d["qt"].ap(), in_=qt)
            nc.sync.dma_start(out=dbg_d["kt"].ap(), in_=kt)
        for m in (5, 6, 7):
            oproj_m(0, m)
        # batch-1 half: only this waits on the tail collective
        normalize(1)
        for m in range(KS):
            oproj_m(1, m)

    nc.compile()
    return nc


_NC = None


def _get_nc():
    global _NC
    if _NC is None:
        _NC = build_nc()
    return _NC


def _rope_perm():
    # per-head deinterleave: evens first then odds, applied to a 128-row slice
    p = []
    for h in range(HPC):
        p.extend(h * DH + np.arange(0, DH, 2))
        p.extend(h * DH + np.arange(1, DH, 2))
    return np.asarray(p)


def make_in_maps(inputs):
    x = np.asarray(inputs["in_features"], np.float32)
    Wq = np.asarray(inputs["Wq"], np.float32)
    Wk = np.asarray(inputs["Wk"], np.float32)
    Wv = np.asarray(inputs["Wv"], np.float32)
    Wo = np.asarray(inputs["Wo"], np.float32)
    pos = np.asarray(inputs["token_positions"]).astype(np.float32)[None, :]

    bf = ml_dtypes.bfloat16
    XT = np.ascontiguousarray(x.reshape(TOK, D).T).astype(bf)
    WoT = np.ascontiguousarray(Wo.T).astype(bf)
    perm = _rope_perm()

    in_maps = []
    for c in range(NCORES):
        rows = slice(c * 128, (c + 1) * 128)
        wq_c = Wq[rows][perm]
        wk_c = Wk[rows][perm]
        wv_c = Wv[rows]
        in_maps.append({
            "xt": XT,
            "wqt": np.ascontiguousarray(wq_c.T).astype(bf),
            "wkt": np.ascontiguousarray(wk_c.T).astype(bf),
            "wvt": np.ascontiguousarray(wv_c.T).astype(bf),
            "wot": WoT,
            "posf": pos,
        })
    return in_maps


def assemble_out(results):
    # core c returns out [D, 512]: cols 0:256 = batch-0 tokens
    # [256c, 256c+256), cols 256:512 = the same positions of batch 1.
    full = np.empty((TOK, D), np.float32)
    for c in range(NCORES):
        chunk = np.asarray(results[c]["out"])  # [D, 512]
        full[256 * c:256 * (c + 1)] = chunk[:, 0:256].T
        full[T + 256 * c:T + 256 * (c + 1)] = chunk[:, 256:512].T
    return full.reshape(B, T, D)


def run(inputs, **kwargs):
    nc = _get_nc()
    res = run_bass_kernel_spmd(
        nc, make_in_maps(inputs), core_ids=list(range(NCORES)), **kwargs
    )
    return assemble_out(res.results), res


def kernel(**inputs) -> np.ndarray:
    out, _ = run(inputs)
    return out


# ---------------------------------------------------------------------------
# Benchmark path: cached jitted executable so repeat executions can be timed
# without retracing/recompiling. Mirrors bass2jax.run_bass_via_pjrt.
# ---------------------------------------------------------------------------
_EXEC = {}


def _build_exec(kind="main"):
    if kind in _EXEC:
        return _EXEC[kind]
    import jax
    from jax.experimental.shard_map import shard_map
    from jax.sharding import Mesh, PartitionSpec

    import concourse.mybir as mybir
    from concourse import bass2jax

    if kind == "main":
        nc = _get_nc()
    elif kind == "dbg":
        nc = build_nc(dbg=True)
    else:
        nc = build_nc(dummy=True)
    bass2jax.install_neuronx_cc_hook()

    partition_name = nc.partition_id_tensor.name if nc.partition_id_tensor else None
    in_names, out_names, out_avals, zero_outs = [], [], [], []
    for alloc in nc.m.functions[0].allocations:
        if not isinstance(alloc, mybir.MemoryLocationSet):
            continue
        name = alloc.memorylocations[0].name
        if alloc.kind == "ExternalInput":
            if name != partition_name:
                in_names.append(name)
        elif alloc.kind == "ExternalOutput":
            out_names.append(name)
            shape = tuple(alloc.tensor_shape)
            dtype = mybir.dt.np(alloc.dtype)
            out_avals.append(jax.core.ShapedArray(shape, dtype))
            zero_outs.append(np.zeros(shape, dtype))
    n_params = len(in_names)
    all_names = list(in_names) + list(out_names)
    if partition_name is not None:
        all_names.append(partition_name)

    def _body(*args):
        outs = bass2jax._bass_exec_p.bind(
            *(list(args) + ([bass2jax.partition_id_tensor()]
                            if partition_name is not None else [])),
            out_avals=tuple(out_avals),
            in_names=tuple(all_names),
            out_names=tuple(out_names),
            lowering_input_output_aliases=(),
            sim_require_finite=True,
            sim_require_nnan=True,
            nc=nc,
        )
        return tuple(outs)

    devices = jax.devices()[:NCORES]
    mesh = Mesh(np.asarray(devices), ("core",))
    nspec = n_params + len(out_names)
    sharded = jax.jit(
        shard_map(
            _body,
            mesh=mesh,
            in_specs=(PartitionSpec("core"),) * nspec,
            out_specs=(PartitionSpec("core"),) * len(out_names),
            check_rep=False,
        ),
        keep_unused=True,
    )
    _EXEC[kind] = (sharded, in_names, out_names, zero_outs, mesh)
    return _EXEC[kind]


def _stage_args(inputs, exec_tuple):
    import jax
    from jax.sharding import NamedSharding, PartitionSpec

    sharded, in_names, out_names, zero_outs, mesh = exec_tuple
    in_maps = make_in_maps(inputs)
    sh = NamedSharding(mesh, PartitionSpec("core"))
    args = []
    for name in in_names:
        cat = np.concatenate([in_maps[c][name] for c in range(NCORES)], axis=0)
        args.append(jax.device_put(cat, sh))
    for z in zero_outs:
        cat = np.concatenate([z] * NCORES, axis=0)
        args.append(jax.device_put(cat, sh))
    return args


def _timed(fn, args, iters):
    import time

    import jax

    jax.block_until_ready(fn(*args))
    best = float("inf")
    for _ in range(iters):
        t0 = time.perf_counter()
        outs = fn(*args)
        jax.block_until_ready(outs)
        best = min(best, time.perf_counter() - t0)
    return outs, best


def run_bench(inputs, iters=10):
    """Returns (output, est_exec_seconds, t_full, t_dummy): times the real
    kernel and a near-empty NEFF with identical I/O; the difference cancels
    the axon dispatch + NEFF-launch overhead."""
    e1 = _build_exec("main")
    args = _stage_args(inputs, e1)
    outs, t_full = _timed(e1[0], args, iters)
    ed = _build_exec("dummy")
    _timed(ed[0], args, iters)
    _, t_dummy = _timed(ed[0], args, iters)

    _, in_names, out_names, zero_outs, mesh = e1
    results = []
    for c in range(NCORES):
        m = {}
        for i, name in enumerate(out_names):
            arr = np.asarray(outs[i])
            per = arr.shape[0] // NCORES
            m[name] = arr[c * per:(c + 1) * per]
        results.append(m)
    return assemble_out(results), max(t_full - t_dummy, 0.0), t_full, t_dummy
